# revision 10
# baseline (speedup 1.0000x reference)
"""MoE FFN (top-2 of 8 experts, SwiGLU) for 8 Trainium2 NeuronCores.

Strategy: load-balanced expert parallelism. The router (tiny [T,H]@[H,E]
matmul + softmax + top-2) runs on host as part of sharding; the 16384
(token, expert) pairs are packed into 8 cores x 2 expert-cells of uniform
capacities (c1, c2) found by a small feasibility search, so every core gets
~2048 pairs instead of the max expert load (~2180). Each cell is bound to
one expert; the host supplies that expert's packed weights as the cell's
weight parameters (shared references, no extra packing). Each core runs a
dense SwiGLU FFN over its cells' tokens in bf16 (fp32 PSUM accumulation),
feature-on-partition / token-on-free-dim, weights streamed chunk-by-chunk
(chunk-major over both cells) so SBUF holds one f-chunk per cell turn.

Per-core device program per (f-chunk fc, cell g), blocks of <=512 tokens:
  g_T[f, t] = sum_i w1[h_i, f]^T @ x_T[h_i, t]        (PSUM accum over h-tiles)
  u_T[f, t] likewise with w2
  h_T[f, t] = silu(g_T + b1) * (u_T + b2)             (ACT + DVE, -> bf16)
  y_T[h, t] = sum_f w3[f, h]^T @ h_T[f, t] + b3       (PSUM accum per f-chunk,
                                                       accumulated in SBUF f32)
At the last chunk the accumulated y is emitted as bf16 and written back with
one fused DMA per block on the (otherwise idle) gpsimd SWDGE queue, so the
write-outs never block the weight-streaming queues. A short burst of warm-up
matmuls on a memset tile flips the PE HAM clock-gate to 8/8 before the first
real data lands, and the prologue DMAs are ordered so the first token block
and first w1/w2 pieces arrive as early as possible.
"""

import numpy as np
import ml_dtypes

E = 8       # experts
K = 2       # top-k
H = 1024    # hidden
F = 4096    # ffn dim
BLK = 512   # max tokens per block (moving free dim of every matmul)
FCH = 512   # f-chunk size (weight streaming granularity); FCH % 128 == 0

NHT = H // 128    # h-tiles
NFCH = F // FCH   # f-chunks
NFT = FCH // 128  # f-tiles per chunk

_BF16 = ml_dtypes.bfloat16

_kernel_cache: dict[object, object] = {}
_last_in_maps = None


def _cell_blocks(c: int, small_first: bool):
    """Decompose a cell capacity into token blocks of <=512."""
    r = c % BLK
    blocks = [BLK] * (c // BLK)
    if r:
        blocks = ([r] + blocks) if small_first else (blocks + [r])
    return blocks


def _plan_cells(loads: list[int]):
    """Find uniform cell capacities (c1 >= c2) and an assignment of experts
    to the 8 c1-cells + 8 c2-cells minimizing cap = c1 + c2.

    Returns (c1, c2, cells1, cells2) where cells1/cells2 are length-8 lists
    of (expert, n_tokens) per core (n_tokens may be 0 for unused cells)."""
    order = sorted(range(E), key=lambda e: -loads[e])

    def try_fit(c1, c2):
        # DFS over experts (desc load): pick (a, b) cells with
        # a*c1 + b*c2 >= load, total a <= 8, b <= 8.
        picks = {}

        def dfs(i, a_left, b_left):
            if i == len(order):
                return True
            L = loads[order[i]]
            cands = []
            for a in range(0, a_left + 1):
                rem = L - a * c1
                b = 0 if rem <= 0 else -(-rem // c2)
                if b <= b_left:
                    cands.append((a + b, a, b))
            cands.sort()
            for _, a, b in cands:
                picks[order[i]] = (a, b)
                if dfs(i + 1, a_left - a, b_left - b):
                    return True
            picks.pop(order[i], None)
            return False

        return picks if dfs(0, E, E) else None

    for cap in range(2048, 2048 + 1024, 16):
        lo = (cap + 1) // 2
        lo = -(-lo // 16) * 16
        for c1 in range(lo, cap - 255, 16):
            c2 = cap - c1
            if c2 < 256 or c2 > c1:
                continue
            picks = try_fit(c1, c2)
            if picks is not None:
                # materialize cells: assign expert cells to cores in order
                cells1, cells2 = [], []
                for e in order:
                    a, b = picks[e]
                    rem = loads[e]
                    for _ in range(a):
                        n = min(rem, c1)
                        cells1.append((e, n))
                        rem -= n
                    for _ in range(b):
                        n = min(rem, c2)
                        cells2.append((e, n))
                        rem -= n
                while len(cells1) < E:
                    cells1.append((0, 0))
                while len(cells2) < E:
                    cells2.append((0, 0))
                return c1, c2, cells1, cells2
    raise RuntimeError("no feasible cell plan found")


def _build(c1: int, c2: int, use_b2: bool):
    """Build the per-core Bass/Tile program for cell capacities (c1, c2)."""
    import concourse.bass as bass  # noqa: F401
    import concourse.tile as tile
    from concourse import bacc, mybir

    bf16 = mybir.dt.bfloat16
    f32 = mybir.dt.float32
    AF = mybir.ActivationFunctionType

    cap = c1 + c2
    blocks = _core_blocks(c1, c2)

    nc = bacc.Bacc("TRN2", target_bir_lowering=False, debug=False, num_devices=E)

    xT = nc.declare_dram_parameter("xT", [128, NHT * cap], bf16, isOutput=False)
    wps = []  # weight params per group: (w1, w2, w3)
    bps = []  # bias params per group: (b1, b3) or (b1, b2, b3)
    for g in range(2):
        w1 = nc.declare_dram_parameter(f"w1{g}", [NFCH, 128, NFT * H], bf16, isOutput=False)
        w2 = nc.declare_dram_parameter(f"w2{g}", [NFCH, 128, NFT * H], bf16, isOutput=False)
        w3 = nc.declare_dram_parameter(f"w3{g}", [NFCH, 128, NFT * H], bf16, isOutput=False)
        wps.append((w1, w2, w3))
        b1 = nc.declare_dram_parameter(f"b1{g}", [128, F // 128], f32, isOutput=False)
        b3 = nc.declare_dram_parameter(f"b3{g}", [128, NHT], f32, isOutput=False)
        if use_b2:
            b2 = nc.declare_dram_parameter(f"b2{g}", [128, F // 128], f32, isOutput=False)
            bps.append((b1, b2, b3))
        else:
            bps.append((b1, b3))
    yT = nc.declare_dram_parameter("yT", [128, NHT * cap], bf16, isOutput=True)

    with tile.TileContext(nc) as tc:
        with (
            tc.tile_pool(name="xp", bufs=1) as xp,
            tc.tile_pool(name="yp", bufs=1) as yp,
            tc.tile_pool(name="wp", bufs=2) as wp,
            tc.tile_pool(name="hp", bufs=2) as hp,
            tc.tile_pool(name="sp", bufs=3) as sp,
            tc.tile_pool(name="pg", bufs=2, space="PSUM") as pg,
            tc.tile_pool(name="pu", bufs=2, space="PSUM") as pu,
            tc.tile_pool(name="py", bufs=2, space="PSUM") as py,
        ):
            op = hp  # write-out tiles share the hp pool (fewer pools)
            # ---- HAM warm-up: keep the PE busy from ~4us so the clock-gate
            # is at 8/8 when the first real matmul issues. No data deps.
            warm = xp.tile([128, 512], bf16, name="warm")
            nc.vector.memset(warm[:], 0)
            psw = py.tile([128, 512], f32, tag="y", name="psw")
            NWARM = 8
            for k in range(NWARM):
                nc.tensor.matmul(
                    psw[:], warm[:, 0:128], warm[:],
                    start=(k == 0), stop=(k == NWARM - 1),
                )

            # ---- resident tiles
            # Tokens (bf16): block-major columns — block at global offset
            # `off` spans cols [NHT*off, NHT*(off+sz)), h-tile i contiguous
            # inside it (col = NHT*off + i*sz + t). Host supplies identical
            # layout: each block is ONE contiguous 2D transfer.
            xall = xp.tile([128, NHT * cap], bf16, name="xall")

            def xsl(i, off, sz):
                base = NHT * off + i * sz
                return xall[:, base:base + sz]

            # f32 accumulator for chunks 0..NFCH-2, h-tile-major columns.
            yall = yp.tile([128, NHT * cap], f32, name="yall")

            def ysl(i, off, sz):
                return yall[:, i * cap + off:i * cap + off + sz]

            # ---- prologue DMAs, critical path first.
            # scalar q: token blocks in processing order (x of first block is
            # the critical path together with w1/w2 chunk-0 pieces on sync).
            for o, s, g in blocks:
                lo, hi = NHT * o, NHT * (o + s)
                nc.scalar.dma_start(xall[:, lo:hi], xT[:, lo:hi])

            # sync q: w1/w2 half-pieces of (chunk0, cell A) interleaved, then
            # biases, then the rest and w3.
            bts = [None, None]
            w1cA = wp.tile([128, NFT * H], bf16, tag="w1", name="w1c")
            w2cA = wp.tile([128, NFT * H], bf16, tag="w2", name="w2c")
            half = (NFT // 2) * H
            nc.sync.dma_start(w1cA[:, 0:half], wps[0][0][0][:, 0:half])
            nc.sync.dma_start(w2cA[:, 0:half], wps[0][1][0][:, 0:half])
            b1t = xp.tile([128, F // 128], f32, name="b1t0")
            nc.sync.dma_start(b1t[:], bps[0][0][:])
            b3t = xp.tile([128, NHT], f32, name="b3t0")
            nc.sync.dma_start(b3t[:], bps[0][-1][:])
            if use_b2:
                b2t = xp.tile([128, F // 128], f32, name="b2t0")
                nc.sync.dma_start(b2t[:], bps[0][1][:])
                bts[0] = (b1t, b2t, b3t)
            else:
                bts[0] = (b1t, b3t)
            nc.sync.dma_start(w1cA[:, half:], wps[0][0][0][:, half:])
            nc.sync.dma_start(w2cA[:, half:], wps[0][1][0][:, half:])
            w3cA = wp.tile([128, NFT * H], bf16, tag="w3", name="w3c")
            nc.sync.dma_start(w3cA[:], wps[0][2][0])
            # cell-B biases
            b1t = xp.tile([128, F // 128], f32, name="b1t1")
            nc.sync.dma_start(b1t[:], bps[1][0][:])
            b3t = xp.tile([128, NHT], f32, name="b3t1")
            nc.sync.dma_start(b3t[:], bps[1][-1][:])
            if use_b2:
                b2t = xp.tile([128, F // 128], f32, name="b2t1")
                nc.sync.dma_start(b2t[:], bps[1][1][:])
                bts[1] = (b1t, b2t, b3t)
            else:
                bts[1] = (b1t, b3t)

            def stage_b(fc, grp, off, sz, ht_tiles, w3t):
                b3t = bts[grp][-1]
                yo = None
                if fc == NFCH - 1:
                    yo = op.tile([128, NHT * sz], bf16, tag="yo", name="yo")
                for i in range(NHT):
                    psy = py.tile([128, sz], f32, tag="y", name="psy")
                    for j in range(NFT):
                        nc.tensor.matmul(
                            psy[:],
                            w3t[:, j * H + i * 128:j * H + (i + 1) * 128],
                            ht_tiles[j][:],
                            start=(j == 0), stop=(j == NFT - 1),
                        )
                    if fc == 0:
                        nc.scalar.activation(
                            ysl(i, off, sz), psy[:], AF.Identity,
                            bias=b3t[:, i:i + 1],
                        )
                    elif fc < NFCH - 1:
                        nc.vector.tensor_add(
                            ysl(i, off, sz), ysl(i, off, sz), psy[:]
                        )
                    else:
                        nc.vector.tensor_add(
                            yo[:, i * sz:(i + 1) * sz], ysl(i, off, sz), psy[:]
                        )
                        if i == NHT // 2 - 1 or i == NHT - 1:
                            # half write-outs as soon as the data is ready;
                            # sync queue is safe here because both groups'
                            # weight loads for this chunk were issued before
                            # any write-out (no queue-order cycle).
                            l = 0 if i < NHT - 1 else (NHT // 2) * sz
                            r = (NHT // 2) * sz if i < NHT - 1 else NHT * sz
                            nc.sync.dma_start(
                                yT[:, NHT * off + l:NHT * off + r],
                                yo[:, l:r],
                            )

            pending = None
            for fc in range(NFCH):
                # load this chunk's weights for BOTH cells up front (tag
                # alternation keeps the bufs=2 double-buffering intact)
                wt = [None, None]
                wt[0] = (w1cA, w2cA, w3cA) if fc == 0 else None
                if wt[0] is None:
                    w1c = wp.tile([128, NFT * H], bf16, tag="w1", name="w1c")
                    nc.sync.dma_start(w1c[:], wps[0][0][fc])
                    w2c = wp.tile([128, NFT * H], bf16, tag="w2", name="w2c")
                    nc.sync.dma_start(w2c[:], wps[0][1][fc])
                    w3c = wp.tile([128, NFT * H], bf16, tag="w3", name="w3c")
                    nc.sync.dma_start(w3c[:], wps[0][2][fc])
                    wt[0] = (w1c, w2c, w3c)
                w1c = wp.tile([128, NFT * H], bf16, tag="w1", name="w1c")
                nc.sync.dma_start(w1c[:], wps[1][0][fc])
                w2c = wp.tile([128, NFT * H], bf16, tag="w2", name="w2c")
                nc.sync.dma_start(w2c[:], wps[1][1][fc])
                w3c = wp.tile([128, NFT * H], bf16, tag="w3", name="w3c")
                nc.sync.dma_start(w3c[:], wps[1][2][fc])
                wt[1] = (w1c, w2c, w3c)

                for grp in range(2):
                    w1c, w2c, w3c = wt[grp]
                    b1t = bts[grp][0]
                    for off, sz, g in blocks:
                        if g != grp:
                            continue
                        # Stage A: h_T[f, tok] = silu(g_T + b1) * (u_T + b2)
                        ht_tiles = []
                        for j in range(NFT):
                            fg = fc * NFT + j
                            psg = pg.tile([128, sz], f32, tag="g", name="psg")
                            for i in range(NHT):
                                base = (j * NHT + i) * 128
                                nc.tensor.matmul(
                                    psg[:], w1c[:, base:base + 128],
                                    xsl(i, off, sz),
                                    start=(i == 0), stop=(i == NHT - 1),
                                )
                            s = sp.tile([128, sz], f32, tag="s", name="stile")
                            nc.scalar.activation(
                                s[:], psg[:], AF.Silu, bias=b1t[:, fg:fg + 1]
                            )
                            psu = pu.tile([128, sz], f32, tag="u", name="psu")
                            for i in range(NHT):
                                base = (j * NHT + i) * 128
                                nc.tensor.matmul(
                                    psu[:], w2c[:, base:base + 128],
                                    xsl(i, off, sz),
                                    start=(i == 0), stop=(i == NHT - 1),
                                )
                            h = hp.tile([128, sz], bf16, tag=f"h{j}", name=f"htile{j}")
                            if use_b2:
                                b2t = bts[grp][1]
                                u2 = sp.tile([128, sz], f32, tag="u2", name="u2tile")
                                nc.scalar.activation(
                                    u2[:], psu[:], AF.Identity,
                                    bias=b2t[:, fg:fg + 1]
                                )
                                nc.vector.tensor_mul(h[:], s[:], u2[:])
                            else:
                                nc.vector.tensor_mul(h[:], s[:], psu[:])
                            ht_tiles.append(h)

                        if pending is not None:
                            stage_b(*pending)
                        pending = (fc, grp, off, sz, ht_tiles, w3c)
            stage_b(*pending)

    nc.finalize()
    return nc


def _route(x2d: np.ndarray, router_w: np.ndarray):
    """Host router: softmax over experts, top-2. Returns per-expert token
    index lists and combine weights."""
    logits = x2d @ router_w                       # [T, E]
    logits -= logits.max(axis=-1, keepdims=True)
    p = np.exp(logits, dtype=np.float32)
    p /= p.sum(axis=-1, keepdims=True)
    order = np.argsort(-p, axis=-1, kind="stable")[:, :K]   # [T, K]
    idx_e, cw_e = [], []
    for e in range(E):
        sel = np.nonzero((order == e).any(axis=1))[0]
        idx_e.append(sel)
        cw_e.append(p[sel, e])
    return idx_e, cw_e


def _pack_w12(w: np.ndarray) -> np.ndarray:
    """[H, F] f32 -> [NFCH, 128, NFT*NHT*128] bf16 with column order (j, i, q):
    chunk c, partition p, f-tile j, h-tile i, col q = w[i*128+p, c*FCH+j*128+q].
    """
    t = np.asarray(w, dtype=np.float32).reshape(NHT, 128, NFCH, NFT, 128)
    t = t.transpose(2, 1, 3, 0, 4)  # [c, p, j, i, q]
    return np.ascontiguousarray(t.astype(_BF16)).reshape(NFCH, 128, NFT * H)


def _pack_w3(w: np.ndarray) -> np.ndarray:
    """[F, H] f32 -> [NFCH, 128, NFT*H] bf16 with column order (j, h):
    chunk c, partition p (= f within f-tile j) -> w[c*FCH+j*128+p, h]."""
    t = np.asarray(w, dtype=np.float32).reshape(NFCH, NFT, 128, H)
    t = t.transpose(0, 2, 1, 3)  # [c, p, j, h]
    return np.ascontiguousarray(t.astype(_BF16)).reshape(NFCH, 128, NFT * H)


def _core_blocks(c1: int, c2: int):
    # cell A: small block first — its x transfer is on the prologue critical
    # path, so a small first block starts the PE several us earlier. cell B:
    # big first, so the final block (whose stage B + write-out is the tail)
    # is the small one.
    blocks = []
    off = 0
    for sz in _cell_blocks(c1, small_first=True):
        blocks.append((off, sz, 0))
        off += sz
    for sz in _cell_blocks(c2, small_first=False):
        blocks.append((off, sz, 1))
        off += sz
    return blocks


def kernel(x, router_w, w1, b1, w2, b2, w3, b3):
    from concourse.bass_utils import run_bass_kernel_spmd

    B, S, _ = x.shape
    T = B * S
    x2d = np.ascontiguousarray(x, dtype=np.float32).reshape(T, H)

    idx_e, cw_e = _route(x2d, np.asarray(router_w, dtype=np.float32))
    loads = [len(i) for i in idx_e]
    c1, c2, cells1, cells2 = _plan_cells(loads)
    cap = c1 + c2

    # token ranges per cell: experts consume their index lists in cell order
    # (cells1 scan order, then cells2) — must match _plan_cells's fill order.
    eoff = [0] * E
    core_cells = [[None, None] for _ in range(E)]
    for g, cells, ccap in ((0, cells1, c1), (1, cells2, c2)):
        for core, (e, n) in enumerate(cells):
            core_cells[core][g] = (e, eoff[e], n)
            eoff[e] += n
    for e in range(E):
        assert eoff[e] == loads[e], (e, eoff[e], loads[e])

    use_b2 = bool(np.any(b2))
    key = (c1, c2, use_b2)
    nc = _kernel_cache.get(key)
    if nc is None:
        nc = _build(c1, c2, use_b2)
        _kernel_cache[key] = nc

    # pack weights once per expert (in_maps share references)
    pw1 = [_pack_w12(w1[e]) for e in range(E)]
    pw2 = [_pack_w12(w2[e]) for e in range(E)]
    pw3 = [_pack_w3(w3[e]) for e in range(E)]
    pb1 = [
        np.ascontiguousarray(
            np.asarray(b1[e], dtype=np.float32).reshape(F // 128, 128).T
        )
        for e in range(E)
    ]
    pb3 = [
        np.ascontiguousarray(
            np.asarray(b3[e], dtype=np.float32).reshape(NHT, 128).T
        )
        for e in range(E)
    ]
    if use_b2:
        pb2 = [
            np.ascontiguousarray(
                np.asarray(b2[e], dtype=np.float32).reshape(F // 128, 128).T
            )
            for e in range(E)
        ]

    blocks = _core_blocks(c1, c2)
    cell_off = (0, c1)

    in_maps = []
    for core in range(E):
        # gather this core's tokens: cell A rows [0, c1), cell B rows [c1, cap)
        xg = np.zeros((cap, H), dtype=np.float32)
        for g in range(2):
            e, st, n = core_cells[core][g]
            if n:
                xg[cell_off[g]:cell_off[g] + n] = x2d[idx_e[e][st:st + n]]
        xb = xg.astype(_BF16)
        xTe = np.concatenate(
            [
                xb[off:off + sz].reshape(sz, NHT, 128)
                .transpose(2, 1, 0).reshape(128, NHT * sz)
                for off, sz, _ in blocks
            ],
            axis=1,
        )
        m = {"xT": np.ascontiguousarray(xTe)}
        for g in range(2):
            e = core_cells[core][g][0]
            m[f"w1{g}"] = pw1[e]
            m[f"w2{g}"] = pw2[e]
            m[f"w3{g}"] = pw3[e]
            m[f"b1{g}"] = pb1[e]
            m[f"b3{g}"] = pb3[e]
            if use_b2:
                m[f"b2{g}"] = pb2[e]
        in_maps.append(m)

    global _last_in_maps
    _last_in_maps = in_maps
    res = run_bass_kernel_spmd(nc, in_maps, core_ids=list(range(E)))

    out = np.zeros((T, H), dtype=np.float32)
    for core in range(E):
        yTe = np.asarray(res.results[core]["yT"], dtype=np.float32)
        for g in range(2):
            e, st, n = core_cells[core][g]
            if not n:
                continue
            co = cell_off[g]
            # per-block unpack: cols NHT*off + i*sz + t
            ye = np.empty((core_cells[core][g][2], H), dtype=np.float32)
            for off, sz, bg in blocks:
                if bg != g:
                    continue
                rel = off - co   # row range of this block within the cell
                if rel >= n:
                    continue
                take = min(sz, n - rel)
                blk = yTe[:, NHT * off:NHT * (off + sz)].reshape(128, NHT, sz)
                ye[rel:rel + take] = (
                    blk[:, :, :take].transpose(2, 1, 0).reshape(take, H)
                )
            idx = idx_e[e][st:st + n]
            out[idx] += ye * cw_e[e][st:st + n][:, None]
    return out.reshape(B, S, H)


# revision 12
# speedup vs baseline: 1.0117x; 1.0117x over previous
"""MoE FFN (top-2 of 8 experts, SwiGLU) for 8 Trainium2 NeuronCores.

Strategy: load-balanced expert parallelism. The router (tiny [T,H]@[H,E]
matmul + softmax + top-2) runs on host as part of sharding; the 16384
(token, expert) pairs are packed into 8 cores x 2 expert-cells of uniform
capacities (c1, c2) found by a small feasibility search, so every core gets
~2048 pairs instead of the max expert load (~2180). Each cell is bound to
one expert; the host supplies that expert's packed weights as the cell's
weight parameters (shared references, no extra packing). Each core runs a
dense SwiGLU FFN over its cells' tokens in bf16 (fp32 PSUM accumulation),
feature-on-partition / token-on-free-dim, weights streamed chunk-by-chunk
(chunk-major over both cells) so SBUF holds one f-chunk per cell turn.

Per-core device program per (f-chunk fc, cell g), blocks of <=512 tokens:
  g_T[f, t] = sum_i w1[h_i, f]^T @ x_T[h_i, t]        (PSUM accum over h-tiles)
  u_T[f, t] likewise with w2
  h_T[f, t] = silu(g_T + b1) * (u_T + b2)             (ACT + DVE, -> bf16)
  y_T[h, t] = sum_f w3[f, h]^T @ h_T[f, t] + b3       (PSUM accum per f-chunk,
                                                       accumulated in SBUF f32)
At the last chunk the accumulated y is emitted as bf16 and written back with
one fused DMA per block on the (otherwise idle) gpsimd SWDGE queue, so the
write-outs never block the weight-streaming queues. A short burst of warm-up
matmuls on a memset tile flips the PE HAM clock-gate to 8/8 before the first
real data lands, and the prologue DMAs are ordered so the first token block
and first w1/w2 pieces arrive as early as possible.
"""

import numpy as np
import ml_dtypes

E = 8       # experts
K = 2       # top-k
H = 1024    # hidden
F = 4096    # ffn dim
BLK = 512   # max tokens per block (moving free dim of every matmul)
FCH = 512   # f-chunk size (weight streaming granularity); FCH % 128 == 0

NHT = H // 128    # h-tiles
NFCH = F // FCH   # f-chunks
NFT = FCH // 128  # f-tiles per chunk

_BF16 = ml_dtypes.bfloat16

_kernel_cache: dict[object, object] = {}
_last_in_maps = None


def _cell_blocks(c: int, small_first: bool):
    """Decompose a cell capacity into token blocks of <=512."""
    r = c % BLK
    blocks = [BLK] * (c // BLK)
    if r:
        blocks = ([r] + blocks) if small_first else (blocks + [r])
    return blocks


def _plan_cells(loads: list[int]):
    """Find uniform cell capacities (c1 >= c2) and an assignment of experts
    to the 8 c1-cells + 8 c2-cells minimizing cap = c1 + c2.

    Returns (c1, c2, cells1, cells2) where cells1/cells2 are length-8 lists
    of (expert, n_tokens) per core (n_tokens may be 0 for unused cells)."""
    order = sorted(range(E), key=lambda e: -loads[e])

    def try_fit(c1, c2):
        # DFS over experts (desc load): pick (a, b) cells with
        # a*c1 + b*c2 >= load, total a <= 8, b <= 8.
        picks = {}

        def dfs(i, a_left, b_left):
            if i == len(order):
                return True
            L = loads[order[i]]
            cands = []
            for a in range(0, a_left + 1):
                rem = L - a * c1
                b = 0 if rem <= 0 else -(-rem // c2)
                if b <= b_left:
                    cands.append((a + b, a, b))
            cands.sort()
            for _, a, b in cands:
                picks[order[i]] = (a, b)
                if dfs(i + 1, a_left - a, b_left - b):
                    return True
            picks.pop(order[i], None)
            return False

        return picks if dfs(0, E, E) else None

    for cap in range(2048, 2048 + 1024, 16):
        lo = (cap + 1) // 2
        lo = -(-lo // 16) * 16
        for c1 in range(lo, cap - 255, 16):
            c2 = cap - c1
            if c2 < 256 or c2 > c1:
                continue
            picks = try_fit(c1, c2)
            if picks is not None:
                # materialize cells: assign expert cells to cores in order
                cells1, cells2 = [], []
                for e in order:
                    a, b = picks[e]
                    rem = loads[e]
                    for _ in range(a):
                        n = min(rem, c1)
                        cells1.append((e, n))
                        rem -= n
                    for _ in range(b):
                        n = min(rem, c2)
                        cells2.append((e, n))
                        rem -= n
                while len(cells1) < E:
                    cells1.append((0, 0))
                while len(cells2) < E:
                    cells2.append((0, 0))
                return c1, c2, cells1, cells2
    raise RuntimeError("no feasible cell plan found")


def _build(c1: int, c2: int, use_b2: bool):
    """Build the per-core Bass/Tile program for cell capacities (c1, c2)."""
    import concourse.bass as bass  # noqa: F401
    import concourse.tile as tile
    from concourse import bacc, mybir

    bf16 = mybir.dt.bfloat16
    f32 = mybir.dt.float32
    AF = mybir.ActivationFunctionType

    cap = c1 + c2
    blocks = _core_blocks(c1, c2)

    nc = bacc.Bacc("TRN2", target_bir_lowering=False, debug=False, num_devices=E)

    xT = nc.declare_dram_parameter("xT", [128, NHT * cap], bf16, isOutput=False)
    wps = []  # weight params per group: (w1, w2, w3)
    bps = []  # bias params per group: (b1, b3) or (b1, b2, b3)
    for g in range(2):
        w1 = nc.declare_dram_parameter(f"w1{g}", [NFCH, 128, NFT * H], bf16, isOutput=False)
        w2 = nc.declare_dram_parameter(f"w2{g}", [NFCH, 128, NFT * H], bf16, isOutput=False)
        w3 = nc.declare_dram_parameter(f"w3{g}", [NFCH, 128, NFT * H], bf16, isOutput=False)
        wps.append((w1, w2, w3))
        b1 = nc.declare_dram_parameter(f"b1{g}", [128, F // 128], f32, isOutput=False)
        b3 = nc.declare_dram_parameter(f"b3{g}", [128, NHT], f32, isOutput=False)
        if use_b2:
            b2 = nc.declare_dram_parameter(f"b2{g}", [128, F // 128], f32, isOutput=False)
            bps.append((b1, b2, b3))
        else:
            bps.append((b1, b3))
    yT = nc.declare_dram_parameter("yT", [128, NHT * cap], bf16, isOutput=True)

    with tile.TileContext(nc) as tc:
        with (
            tc.tile_pool(name="xp", bufs=1) as xp,
            tc.tile_pool(name="yp", bufs=1) as yp,
            tc.tile_pool(name="wp", bufs=2) as wp,
            tc.tile_pool(name="hp", bufs=2) as hp,
            tc.tile_pool(name="sp", bufs=3) as sp,
            tc.tile_pool(name="pg", bufs=2, space="PSUM") as pg,
            tc.tile_pool(name="pu", bufs=2, space="PSUM") as pu,
            tc.tile_pool(name="py", bufs=2, space="PSUM") as py,
        ):
            op = hp  # write-out tiles share the hp pool (fewer pools)
            # ---- HAM warm-up: keep the PE busy from ~4us so the clock-gate
            # is at 8/8 when the first real matmul issues. No data deps.
            warm = xp.tile([128, 512], bf16, name="warm")
            nc.vector.memset(warm[:], 0)
            psw = py.tile([128, 512], f32, tag="y", name="psw")
            NWARM = 16
            for k in range(NWARM):
                nc.tensor.matmul(
                    psw[:], warm[:, 0:128], warm[:],
                    start=(k == 0), stop=(k == NWARM - 1),
                )

            # ---- resident tiles
            # Tokens (bf16): block-major columns — block at global offset
            # `off` spans cols [NHT*off, NHT*(off+sz)), h-tile i contiguous
            # inside it (col = NHT*off + i*sz + t). Host supplies identical
            # layout: each block is ONE contiguous 2D transfer.
            xall = xp.tile([128, NHT * cap], bf16, name="xall")

            def xsl(i, off, sz):
                base = NHT * off + i * sz
                return xall[:, base:base + sz]

            # f32 accumulator for chunks 0..NFCH-2, h-tile-major columns.
            yall = yp.tile([128, NHT * cap], f32, name="yall")

            def ysl(i, off, sz):
                return yall[:, i * cap + off:i * cap + off + sz]

            # ---- prologue DMAs, critical path first.
            # scalar q: token blocks in processing order (x of first block is
            # the critical path together with w1/w2 chunk-0 pieces on sync).
            for o, s, g in blocks:
                lo, hi = NHT * o, NHT * (o + s)
                nc.scalar.dma_start(xall[:, lo:hi], xT[:, lo:hi])

            # sync q: w1/w2 half-pieces of (chunk0, cell A) interleaved, then
            # biases, then the rest and w3.
            bts = [None, None]
            w1cA = wp.tile([128, NFT * H], bf16, tag="w1", name="w1c")
            w2cA = wp.tile([128, NFT * H], bf16, tag="w2", name="w2c")
            half = (NFT // 2) * H
            nc.sync.dma_start(w1cA[:, 0:half], wps[0][0][0][:, 0:half])
            nc.sync.dma_start(w2cA[:, 0:half], wps[0][1][0][:, 0:half])
            b1t = xp.tile([128, F // 128], f32, name="b1t0")
            nc.sync.dma_start(b1t[:], bps[0][0][:])
            b3t = xp.tile([128, NHT], f32, name="b3t0")
            nc.sync.dma_start(b3t[:], bps[0][-1][:])
            if use_b2:
                b2t = xp.tile([128, F // 128], f32, name="b2t0")
                nc.sync.dma_start(b2t[:], bps[0][1][:])
                bts[0] = (b1t, b2t, b3t)
            else:
                bts[0] = (b1t, b3t)
            nc.sync.dma_start(w1cA[:, half:], wps[0][0][0][:, half:])
            nc.sync.dma_start(w2cA[:, half:], wps[0][1][0][:, half:])
            w3cA = wp.tile([128, NFT * H], bf16, tag="w3", name="w3c")
            nc.sync.dma_start(w3cA[:], wps[0][2][0])
            # cell-B biases
            b1t = xp.tile([128, F // 128], f32, name="b1t1")
            nc.sync.dma_start(b1t[:], bps[1][0][:])
            b3t = xp.tile([128, NHT], f32, name="b3t1")
            nc.sync.dma_start(b3t[:], bps[1][-1][:])
            if use_b2:
                b2t = xp.tile([128, F // 128], f32, name="b2t1")
                nc.sync.dma_start(b2t[:], bps[1][1][:])
                bts[1] = (b1t, b2t, b3t)
            else:
                bts[1] = (b1t, b3t)

            def stage_b(fc, grp, off, sz, ht_tiles, w3t):
                b3t = bts[grp][-1]
                yo = None
                if fc == NFCH - 1:
                    yo = op.tile([128, NHT * sz], bf16, tag="yo", name="yo")
                for i in range(NHT):
                    psy = py.tile([128, sz], f32, tag="y", name="psy")
                    for j in range(NFT):
                        nc.tensor.matmul(
                            psy[:],
                            w3t[:, j * H + i * 128:j * H + (i + 1) * 128],
                            ht_tiles[j][:],
                            start=(j == 0), stop=(j == NFT - 1),
                        )
                    if fc == 0:
                        nc.scalar.activation(
                            ysl(i, off, sz), psy[:], AF.Identity,
                            bias=b3t[:, i:i + 1],
                        )
                    elif fc < NFCH - 1:
                        nc.vector.tensor_add(
                            ysl(i, off, sz), ysl(i, off, sz), psy[:]
                        )
                    else:
                        nc.vector.tensor_add(
                            yo[:, i * sz:(i + 1) * sz], ysl(i, off, sz), psy[:]
                        )
                        if i == NHT // 2 - 1 or i == NHT - 1:
                            # half write-outs as soon as the data is ready;
                            # sync queue is safe here because both groups'
                            # weight loads for this chunk were issued before
                            # any write-out (no queue-order cycle).
                            l = 0 if i < NHT - 1 else (NHT // 2) * sz
                            r = (NHT // 2) * sz if i < NHT - 1 else NHT * sz
                            nc.sync.dma_start(
                                yT[:, NHT * off + l:NHT * off + r],
                                yo[:, l:r],
                            )

            pending = None
            for fc in range(NFCH):
                # load this chunk's weights for BOTH cells up front (tag
                # alternation keeps the bufs=2 double-buffering intact)
                wt = [None, None]
                wt[0] = (w1cA, w2cA, w3cA) if fc == 0 else None
                if wt[0] is None:
                    w1c = wp.tile([128, NFT * H], bf16, tag="w1", name="w1c")
                    nc.sync.dma_start(w1c[:], wps[0][0][fc])
                    w2c = wp.tile([128, NFT * H], bf16, tag="w2", name="w2c")
                    nc.sync.dma_start(w2c[:], wps[0][1][fc])
                    w3c = wp.tile([128, NFT * H], bf16, tag="w3", name="w3c")
                    nc.sync.dma_start(w3c[:], wps[0][2][fc])
                    wt[0] = (w1c, w2c, w3c)
                w1c = wp.tile([128, NFT * H], bf16, tag="w1", name="w1c")
                nc.sync.dma_start(w1c[:], wps[1][0][fc])
                w2c = wp.tile([128, NFT * H], bf16, tag="w2", name="w2c")
                nc.sync.dma_start(w2c[:], wps[1][1][fc])
                w3c = wp.tile([128, NFT * H], bf16, tag="w3", name="w3c")
                nc.sync.dma_start(w3c[:], wps[1][2][fc])
                wt[1] = (w1c, w2c, w3c)

                for grp in range(2):
                    w1c, w2c, w3c = wt[grp]
                    b1t = bts[grp][0]
                    for off, sz, g in blocks:
                        if g != grp:
                            continue
                        # Stage A: h_T[f, tok] = silu(g_T + b1) * (u_T + b2)
                        ht_tiles = []
                        for j in range(NFT):
                            fg = fc * NFT + j
                            psg = pg.tile([128, sz], f32, tag="g", name="psg")
                            for i in range(NHT):
                                base = (j * NHT + i) * 128
                                nc.tensor.matmul(
                                    psg[:], w1c[:, base:base + 128],
                                    xsl(i, off, sz),
                                    start=(i == 0), stop=(i == NHT - 1),
                                )
                            s = sp.tile([128, sz], f32, tag="s", name="stile")
                            nc.scalar.activation(
                                s[:], psg[:], AF.Silu, bias=b1t[:, fg:fg + 1]
                            )
                            psu = pu.tile([128, sz], f32, tag="u", name="psu")
                            for i in range(NHT):
                                base = (j * NHT + i) * 128
                                nc.tensor.matmul(
                                    psu[:], w2c[:, base:base + 128],
                                    xsl(i, off, sz),
                                    start=(i == 0), stop=(i == NHT - 1),
                                )
                            h = hp.tile([128, sz], bf16, tag=f"h{j}", name=f"htile{j}")
                            if use_b2:
                                b2t = bts[grp][1]
                                u2 = sp.tile([128, sz], f32, tag="u2", name="u2tile")
                                nc.scalar.activation(
                                    u2[:], psu[:], AF.Identity,
                                    bias=b2t[:, fg:fg + 1]
                                )
                                nc.vector.tensor_mul(h[:], s[:], u2[:])
                            else:
                                nc.vector.tensor_mul(h[:], s[:], psu[:])
                            ht_tiles.append(h)

                        if pending is not None:
                            stage_b(*pending)
                        pending = (fc, grp, off, sz, ht_tiles, w3c)
            stage_b(*pending)

    nc.finalize()
    return nc


def _route(x2d: np.ndarray, router_w: np.ndarray):
    """Host router: softmax over experts, top-2. Returns per-expert token
    index lists and combine weights."""
    logits = x2d @ router_w                       # [T, E]
    logits -= logits.max(axis=-1, keepdims=True)
    p = np.exp(logits, dtype=np.float32)
    p /= p.sum(axis=-1, keepdims=True)
    order = np.argsort(-p, axis=-1, kind="stable")[:, :K]   # [T, K]
    idx_e, cw_e = [], []
    for e in range(E):
        sel = np.nonzero((order == e).any(axis=1))[0]
        idx_e.append(sel)
        cw_e.append(p[sel, e])
    return idx_e, cw_e


def _pack_w12(w: np.ndarray) -> np.ndarray:
    """[H, F] f32 -> [NFCH, 128, NFT*NHT*128] bf16 with column order (j, i, q):
    chunk c, partition p, f-tile j, h-tile i, col q = w[i*128+p, c*FCH+j*128+q].
    """
    t = np.asarray(w, dtype=np.float32).reshape(NHT, 128, NFCH, NFT, 128)
    t = t.transpose(2, 1, 3, 0, 4)  # [c, p, j, i, q]
    return np.ascontiguousarray(t.astype(_BF16)).reshape(NFCH, 128, NFT * H)


def _pack_w3(w: np.ndarray) -> np.ndarray:
    """[F, H] f32 -> [NFCH, 128, NFT*H] bf16 with column order (j, h):
    chunk c, partition p (= f within f-tile j) -> w[c*FCH+j*128+p, h]."""
    t = np.asarray(w, dtype=np.float32).reshape(NFCH, NFT, 128, H)
    t = t.transpose(0, 2, 1, 3)  # [c, p, j, h]
    return np.ascontiguousarray(t.astype(_BF16)).reshape(NFCH, 128, NFT * H)


def _core_blocks(c1: int, c2: int):
    # big blocks first in both cells: a full first block keeps the PE
    # streaming once its x lands (a small first block makes the ramp dribble
    # at DMA pace), and big-first in cell B leaves a small final block whose
    # stage B + write-out form the kernel tail.
    blocks = []
    off = 0
    for sz in _cell_blocks(c1, small_first=False):
        blocks.append((off, sz, 0))
        off += sz
    for sz in _cell_blocks(c2, small_first=False):
        blocks.append((off, sz, 1))
        off += sz
    return blocks


def kernel(x, router_w, w1, b1, w2, b2, w3, b3):
    from concourse.bass_utils import run_bass_kernel_spmd

    B, S, _ = x.shape
    T = B * S
    x2d = np.ascontiguousarray(x, dtype=np.float32).reshape(T, H)

    idx_e, cw_e = _route(x2d, np.asarray(router_w, dtype=np.float32))
    loads = [len(i) for i in idx_e]
    c1, c2, cells1, cells2 = _plan_cells(loads)
    cap = c1 + c2

    # token ranges per cell: experts consume their index lists in cell order
    # (cells1 scan order, then cells2) — must match _plan_cells's fill order.
    eoff = [0] * E
    core_cells = [[None, None] for _ in range(E)]
    for g, cells, ccap in ((0, cells1, c1), (1, cells2, c2)):
        for core, (e, n) in enumerate(cells):
            core_cells[core][g] = (e, eoff[e], n)
            eoff[e] += n
    for e in range(E):
        assert eoff[e] == loads[e], (e, eoff[e], loads[e])

    use_b2 = bool(np.any(b2))
    key = (c1, c2, use_b2)
    nc = _kernel_cache.get(key)
    if nc is None:
        nc = _build(c1, c2, use_b2)
        _kernel_cache[key] = nc

    # pack weights once per expert (in_maps share references)
    pw1 = [_pack_w12(w1[e]) for e in range(E)]
    pw2 = [_pack_w12(w2[e]) for e in range(E)]
    pw3 = [_pack_w3(w3[e]) for e in range(E)]
    pb1 = [
        np.ascontiguousarray(
            np.asarray(b1[e], dtype=np.float32).reshape(F // 128, 128).T
        )
        for e in range(E)
    ]
    pb3 = [
        np.ascontiguousarray(
            np.asarray(b3[e], dtype=np.float32).reshape(NHT, 128).T
        )
        for e in range(E)
    ]
    if use_b2:
        pb2 = [
            np.ascontiguousarray(
                np.asarray(b2[e], dtype=np.float32).reshape(F // 128, 128).T
            )
            for e in range(E)
        ]

    blocks = _core_blocks(c1, c2)
    cell_off = (0, c1)

    in_maps = []
    for core in range(E):
        # gather this core's tokens: cell A rows [0, c1), cell B rows [c1, cap)
        xg = np.zeros((cap, H), dtype=np.float32)
        for g in range(2):
            e, st, n = core_cells[core][g]
            if n:
                xg[cell_off[g]:cell_off[g] + n] = x2d[idx_e[e][st:st + n]]
        xb = xg.astype(_BF16)
        xTe = np.concatenate(
            [
                xb[off:off + sz].reshape(sz, NHT, 128)
                .transpose(2, 1, 0).reshape(128, NHT * sz)
                for off, sz, _ in blocks
            ],
            axis=1,
        )
        m = {"xT": np.ascontiguousarray(xTe)}
        for g in range(2):
            e = core_cells[core][g][0]
            m[f"w1{g}"] = pw1[e]
            m[f"w2{g}"] = pw2[e]
            m[f"w3{g}"] = pw3[e]
            m[f"b1{g}"] = pb1[e]
            m[f"b3{g}"] = pb3[e]
            if use_b2:
                m[f"b2{g}"] = pb2[e]
        in_maps.append(m)

    global _last_in_maps
    _last_in_maps = in_maps
    res = run_bass_kernel_spmd(nc, in_maps, core_ids=list(range(E)))

    out = np.zeros((T, H), dtype=np.float32)
    for core in range(E):
        yTe = np.asarray(res.results[core]["yT"], dtype=np.float32)
        for g in range(2):
            e, st, n = core_cells[core][g]
            if not n:
                continue
            co = cell_off[g]
            # per-block unpack: cols NHT*off + i*sz + t
            ye = np.empty((core_cells[core][g][2], H), dtype=np.float32)
            for off, sz, bg in blocks:
                if bg != g:
                    continue
                rel = off - co   # row range of this block within the cell
                if rel >= n:
                    continue
                take = min(sz, n - rel)
                blk = yTe[:, NHT * off:NHT * (off + sz)].reshape(128, NHT, sz)
                ye[rel:rel + take] = (
                    blk[:, :, :take].transpose(2, 1, 0).reshape(take, H)
                )
            idx = idx_e[e][st:st + n]
            out[idx] += ye * cw_e[e][st:st + n][:, None]
    return out.reshape(B, S, H)


# revision 15
# speedup vs baseline: 1.0141x; 1.0024x over previous
"""MoE FFN (top-2 of 8 experts, SwiGLU) for 8 Trainium2 NeuronCores.

Strategy: load-balanced expert parallelism. The router (tiny [T,H]@[H,E]
matmul + softmax + top-2) runs on host as part of sharding; the 16384
(token, expert) pairs are packed into 8 cores x 2 expert-cells of uniform
capacities (c1, c2) found by a small feasibility search, so every core gets
~2048 pairs instead of the max expert load (~2180). Each cell is bound to
one expert; the host supplies that expert's packed weights as the cell's
weight parameters (shared references, no extra packing). Each core runs a
dense SwiGLU FFN over its cells' tokens in bf16 (fp32 PSUM accumulation),
feature-on-partition / token-on-free-dim, weights streamed chunk-by-chunk
(chunk-major over both cells) so SBUF holds one f-chunk per cell turn.

Per-core device program per (f-chunk fc, cell g), blocks of <=512 tokens:
  g_T[f, t] = sum_i w1[h_i, f]^T @ x_T[h_i, t]        (PSUM accum over h-tiles)
  u_T[f, t] likewise with w2
  h_T[f, t] = silu(g_T + b1) * (u_T + b2)             (ACT + DVE, -> bf16)
  y_T[h, t] = sum_f w3[f, h]^T @ h_T[f, t] + b3       (PSUM accum per f-chunk,
                                                       accumulated in SBUF f32)
At the last chunk the accumulated y is emitted as bf16 and written back with
one fused DMA per block on the (otherwise idle) gpsimd SWDGE queue, so the
write-outs never block the weight-streaming queues. A short burst of warm-up
matmuls on a memset tile flips the PE HAM clock-gate to 8/8 before the first
real data lands, and the prologue DMAs are ordered so the first token block
and first w1/w2 pieces arrive as early as possible.
"""

import numpy as np
import ml_dtypes

E = 8       # experts
K = 2       # top-k
H = 1024    # hidden
F = 4096    # ffn dim
BLK = 512   # max tokens per block (moving free dim of every matmul)
FCH = 512   # f-chunk size (weight streaming granularity); FCH % 128 == 0

NHT = H // 128    # h-tiles
NFCH = F // FCH   # f-chunks
NFT = FCH // 128  # f-tiles per chunk

_BF16 = ml_dtypes.bfloat16

_kernel_cache: dict[object, object] = {}
_last_in_maps = None


def _cell_blocks(c: int, small_first: bool):
    """Decompose a cell capacity into token blocks of <=512."""
    r = c % BLK
    blocks = [BLK] * (c // BLK)
    if r:
        blocks = ([r] + blocks) if small_first else (blocks + [r])
    return blocks


def _plan_cells(loads: list[int]):
    """Find uniform cell capacities (c1 >= c2) and an assignment of experts
    to the 8 c1-cells + 8 c2-cells minimizing cap = c1 + c2.

    Returns (c1, c2, cells1, cells2) where cells1/cells2 are length-8 lists
    of (expert, n_tokens) per core (n_tokens may be 0 for unused cells)."""
    order = sorted(range(E), key=lambda e: -loads[e])

    def try_fit(c1, c2):
        # DFS over experts (desc load): pick (a, b) cells with
        # a*c1 + b*c2 >= load, total a <= 8, b <= 8.
        picks = {}

        def dfs(i, a_left, b_left):
            if i == len(order):
                return True
            L = loads[order[i]]
            cands = []
            for a in range(0, a_left + 1):
                rem = L - a * c1
                b = 0 if rem <= 0 else -(-rem // c2)
                if b <= b_left:
                    cands.append((a + b, a, b))
            cands.sort()
            for _, a, b in cands:
                picks[order[i]] = (a, b)
                if dfs(i + 1, a_left - a, b_left - b):
                    return True
            picks.pop(order[i], None)
            return False

        return picks if dfs(0, E, E) else None

    for cap in range(2048, 2048 + 1024, 16):
        lo = (cap + 1) // 2
        lo = -(-lo // 16) * 16
        for c1 in range(lo, cap - 255, 16):
            c2 = cap - c1
            if c2 < 256 or c2 > c1:
                continue
            picks = try_fit(c1, c2)
            if picks is not None:
                # materialize cells: assign expert cells to cores in order
                cells1, cells2 = [], []
                for e in order:
                    a, b = picks[e]
                    rem = loads[e]
                    for _ in range(a):
                        n = min(rem, c1)
                        cells1.append((e, n))
                        rem -= n
                    for _ in range(b):
                        n = min(rem, c2)
                        cells2.append((e, n))
                        rem -= n
                while len(cells1) < E:
                    cells1.append((0, 0))
                while len(cells2) < E:
                    cells2.append((0, 0))
                return c1, c2, cells1, cells2
    raise RuntimeError("no feasible cell plan found")


def _build(c1: int, c2: int, use_b2: bool):
    """Build the per-core Bass/Tile program for cell capacities (c1, c2)."""
    import concourse.bass as bass  # noqa: F401
    import concourse.tile as tile
    from concourse import bacc, mybir

    bf16 = mybir.dt.bfloat16
    f32 = mybir.dt.float32
    AF = mybir.ActivationFunctionType

    cap = c1 + c2
    blocks = _core_blocks(c1, c2)

    nc = bacc.Bacc("TRN2", target_bir_lowering=False, debug=False, num_devices=E)

    xT = nc.declare_dram_parameter("xT", [128, NHT * cap], bf16, isOutput=False)
    wps = []  # weight params per group: (w1, w2, w3)
    bps = []  # bias params per group: (b1, b3) or (b1, b2, b3)
    for g in range(2):
        w1 = nc.declare_dram_parameter(f"w1{g}", [NFCH, 128, NFT * H], bf16, isOutput=False)
        w2 = nc.declare_dram_parameter(f"w2{g}", [NFCH, 128, NFT * H], bf16, isOutput=False)
        w3 = nc.declare_dram_parameter(f"w3{g}", [NFCH, 128, NFT * H], bf16, isOutput=False)
        wps.append((w1, w2, w3))
        b1 = nc.declare_dram_parameter(f"b1{g}", [128, F // 128], f32, isOutput=False)
        b3 = nc.declare_dram_parameter(f"b3{g}", [128, NHT], f32, isOutput=False)
        if use_b2:
            b2 = nc.declare_dram_parameter(f"b2{g}", [128, F // 128], f32, isOutput=False)
            bps.append((b1, b2, b3))
        else:
            bps.append((b1, b3))
    yT = nc.declare_dram_parameter("yT", [128, NHT * cap], bf16, isOutput=True)

    with tile.TileContext(nc) as tc:
        with (
            tc.tile_pool(name="xp", bufs=1) as xp,
            tc.tile_pool(name="yp", bufs=1) as yp,
            tc.tile_pool(name="wp", bufs=2) as wp,
            tc.tile_pool(name="hp", bufs=2) as hp,
            tc.tile_pool(name="sp", bufs=3) as sp,
            tc.tile_pool(name="pg", bufs=2, space="PSUM") as pg,
            tc.tile_pool(name="pu", bufs=2, space="PSUM") as pu,
            tc.tile_pool(name="py", bufs=2, space="PSUM") as py,
        ):
            op = hp  # write-out tiles share the hp pool (fewer pools)
            # ---- HAM warm-up: keep the PE busy from ~4us so the clock-gate
            # is at 8/8 when the first real matmul issues. No data deps.
            warm = xp.tile([128, 512], bf16, name="warm")
            nc.vector.memset(warm[:], 0)
            psw = py.tile([128, 512], f32, tag="y", name="psw")
            NWARM = 10
            for k in range(NWARM):
                nc.tensor.matmul(
                    psw[:], warm[:, 0:128], warm[:],
                    start=(k == 0), stop=(k == NWARM - 1),
                )

            # ---- resident tiles
            # Tokens (bf16): block-major columns — block at global offset
            # `off` spans cols [NHT*off, NHT*(off+sz)), h-tile i contiguous
            # inside it (col = NHT*off + i*sz + t). Host supplies identical
            # layout: each block is ONE contiguous 2D transfer.
            xall = xp.tile([128, NHT * cap], bf16, name="xall")

            def xsl(i, off, sz):
                base = NHT * off + i * sz
                return xall[:, base:base + sz]

            # f32 accumulator for chunks 0..NFCH-2, h-tile-major columns.
            yall = yp.tile([128, NHT * cap], f32, name="yall")

            def ysl(i, off, sz):
                return yall[:, i * cap + off:i * cap + off + sz]

            # ---- prologue DMAs. The first matmul group needs x of block 0
            # AND w1's first piece; both queues pull concurrently from a
            # shared per-core DMA bandwidth pool, so split x block 0 across
            # both queues and interleave the w1/w2 chunk-0 quarter pieces so
            # the PE can start ~2us after the critical 1.5MB lands.
            o0, s0, _ = blocks[0]
            mid = NHT * o0 + (NHT // 2) * s0
            end0 = NHT * (o0 + s0)
            # scalar q: x block-0 low half, then the remaining blocks
            nc.scalar.dma_start(xall[:, NHT * o0:mid], xT[:, NHT * o0:mid])
            for o, s, g in blocks[1:]:
                lo, hi = NHT * o, NHT * (o + s)
                nc.scalar.dma_start(xall[:, lo:hi], xT[:, lo:hi])

            # sync q: w1 first quarter, x block-0 high half, w2 first
            # quarter, biases, remaining quarters, w3.
            bts = [None, None]
            w1cA = wp.tile([128, NFT * H], bf16, tag="w1", name="w1c")
            w2cA = wp.tile([128, NFT * H], bf16, tag="w2", name="w2c")
            nc.sync.dma_start(w1cA[:, 0:H], wps[0][0][0][:, 0:H])
            nc.sync.dma_start(xall[:, mid:end0], xT[:, mid:end0])
            nc.sync.dma_start(w2cA[:, 0:H], wps[0][1][0][:, 0:H])
            b1t = xp.tile([128, F // 128], f32, name="b1t0")
            nc.sync.dma_start(b1t[:], bps[0][0][:])
            b3t = xp.tile([128, NHT], f32, name="b3t0")
            nc.sync.dma_start(b3t[:], bps[0][-1][:])
            if use_b2:
                b2t = xp.tile([128, F // 128], f32, name="b2t0")
                nc.sync.dma_start(b2t[:], bps[0][1][:])
                bts[0] = (b1t, b2t, b3t)
            else:
                bts[0] = (b1t, b3t)
            for j in range(1, NFT):
                jsl = slice(j * H, (j + 1) * H)
                nc.sync.dma_start(w1cA[:, jsl], wps[0][0][0][:, jsl])
                nc.sync.dma_start(w2cA[:, jsl], wps[0][1][0][:, jsl])
            w3cA = wp.tile([128, NFT * H], bf16, tag="w3", name="w3c")
            nc.sync.dma_start(w3cA[:], wps[0][2][0])
            # cell-B biases
            b1t = xp.tile([128, F // 128], f32, name="b1t1")
            nc.sync.dma_start(b1t[:], bps[1][0][:])
            b3t = xp.tile([128, NHT], f32, name="b3t1")
            nc.sync.dma_start(b3t[:], bps[1][-1][:])
            if use_b2:
                b2t = xp.tile([128, F // 128], f32, name="b2t1")
                nc.sync.dma_start(b2t[:], bps[1][1][:])
                bts[1] = (b1t, b2t, b3t)
            else:
                bts[1] = (b1t, b3t)

            def stage_b(fc, grp, off, sz, ht_tiles, w3t):
                b3t = bts[grp][-1]
                yo = None
                if fc == NFCH - 1:
                    yo = op.tile([128, NHT * sz], bf16, tag="yo", name="yo")
                for i in range(NHT):
                    psy = py.tile([128, sz], f32, tag="y", name="psy")
                    for j in range(NFT):
                        nc.tensor.matmul(
                            psy[:],
                            w3t[:, j * H + i * 128:j * H + (i + 1) * 128],
                            ht_tiles[j][:],
                            start=(j == 0), stop=(j == NFT - 1),
                        )
                    if fc == 0:
                        nc.scalar.activation(
                            ysl(i, off, sz), psy[:], AF.Identity,
                            bias=b3t[:, i:i + 1],
                        )
                    elif fc < NFCH - 1:
                        nc.vector.tensor_add(
                            ysl(i, off, sz), ysl(i, off, sz), psy[:]
                        )
                    else:
                        nc.vector.tensor_add(
                            yo[:, i * sz:(i + 1) * sz], ysl(i, off, sz), psy[:]
                        )
                        if i % 2 == 1:
                            # write out per pair of h-tiles as soon as the
                            # data is ready; sync queue is safe here because
                            # both groups' weight loads for this chunk were
                            # issued before any write-out (no cycle).
                            l, r = (i - 1) * sz, (i + 1) * sz
                            nc.sync.dma_start(
                                yT[:, NHT * off + l:NHT * off + r],
                                yo[:, l:r],
                            )

            pending = None
            for fc in range(NFCH):
                # load this chunk's weights for BOTH cells up front (tag
                # alternation keeps the bufs=2 double-buffering intact)
                wt = [None, None]
                wt[0] = (w1cA, w2cA, w3cA) if fc == 0 else None
                if wt[0] is None:
                    w1c = wp.tile([128, NFT * H], bf16, tag="w1", name="w1c")
                    nc.sync.dma_start(w1c[:], wps[0][0][fc])
                    w2c = wp.tile([128, NFT * H], bf16, tag="w2", name="w2c")
                    nc.sync.dma_start(w2c[:], wps[0][1][fc])
                    w3c = wp.tile([128, NFT * H], bf16, tag="w3", name="w3c")
                    nc.sync.dma_start(w3c[:], wps[0][2][fc])
                    wt[0] = (w1c, w2c, w3c)
                w1c = wp.tile([128, NFT * H], bf16, tag="w1", name="w1c")
                nc.sync.dma_start(w1c[:], wps[1][0][fc])
                w2c = wp.tile([128, NFT * H], bf16, tag="w2", name="w2c")
                nc.sync.dma_start(w2c[:], wps[1][1][fc])
                w3c = wp.tile([128, NFT * H], bf16, tag="w3", name="w3c")
                nc.sync.dma_start(w3c[:], wps[1][2][fc])
                wt[1] = (w1c, w2c, w3c)

                for grp in range(2):
                    w1c, w2c, w3c = wt[grp]
                    b1t = bts[grp][0]
                    for off, sz, g in blocks:
                        if g != grp:
                            continue
                        # Stage A: h_T[f, tok] = silu(g_T + b1) * (u_T + b2)
                        ht_tiles = []
                        for j in range(NFT):
                            fg = fc * NFT + j
                            psg = pg.tile([128, sz], f32, tag="g", name="psg")
                            for i in range(NHT):
                                base = (j * NHT + i) * 128
                                nc.tensor.matmul(
                                    psg[:], w1c[:, base:base + 128],
                                    xsl(i, off, sz),
                                    start=(i == 0), stop=(i == NHT - 1),
                                )
                            s = sp.tile([128, sz], f32, tag="s", name="stile")
                            nc.scalar.activation(
                                s[:], psg[:], AF.Silu, bias=b1t[:, fg:fg + 1]
                            )
                            psu = pu.tile([128, sz], f32, tag="u", name="psu")
                            for i in range(NHT):
                                base = (j * NHT + i) * 128
                                nc.tensor.matmul(
                                    psu[:], w2c[:, base:base + 128],
                                    xsl(i, off, sz),
                                    start=(i == 0), stop=(i == NHT - 1),
                                )
                            h = hp.tile([128, sz], bf16, tag=f"h{j}", name=f"htile{j}")
                            if use_b2:
                                b2t = bts[grp][1]
                                u2 = sp.tile([128, sz], f32, tag="u2", name="u2tile")
                                nc.scalar.activation(
                                    u2[:], psu[:], AF.Identity,
                                    bias=b2t[:, fg:fg + 1]
                                )
                                nc.vector.tensor_mul(h[:], s[:], u2[:])
                            else:
                                nc.vector.tensor_mul(h[:], s[:], psu[:])
                            ht_tiles.append(h)

                        if pending is not None:
                            stage_b(*pending)
                        pending = (fc, grp, off, sz, ht_tiles, w3c)
            stage_b(*pending)

    nc.finalize()
    return nc


def _route(x2d: np.ndarray, router_w: np.ndarray):
    """Host router: softmax over experts, top-2. Returns per-expert token
    index lists and combine weights."""
    logits = x2d @ router_w                       # [T, E]
    logits -= logits.max(axis=-1, keepdims=True)
    p = np.exp(logits, dtype=np.float32)
    p /= p.sum(axis=-1, keepdims=True)
    order = np.argsort(-p, axis=-1, kind="stable")[:, :K]   # [T, K]
    idx_e, cw_e = [], []
    for e in range(E):
        sel = np.nonzero((order == e).any(axis=1))[0]
        idx_e.append(sel)
        cw_e.append(p[sel, e])
    return idx_e, cw_e


def _pack_w12(w: np.ndarray) -> np.ndarray:
    """[H, F] f32 -> [NFCH, 128, NFT*NHT*128] bf16 with column order (j, i, q):
    chunk c, partition p, f-tile j, h-tile i, col q = w[i*128+p, c*FCH+j*128+q].
    """
    t = np.asarray(w, dtype=np.float32).reshape(NHT, 128, NFCH, NFT, 128)
    t = t.transpose(2, 1, 3, 0, 4)  # [c, p, j, i, q]
    return np.ascontiguousarray(t.astype(_BF16)).reshape(NFCH, 128, NFT * H)


def _pack_w3(w: np.ndarray) -> np.ndarray:
    """[F, H] f32 -> [NFCH, 128, NFT*H] bf16 with column order (j, h):
    chunk c, partition p (= f within f-tile j) -> w[c*FCH+j*128+p, h]."""
    t = np.asarray(w, dtype=np.float32).reshape(NFCH, NFT, 128, H)
    t = t.transpose(0, 2, 1, 3)  # [c, p, j, h]
    return np.ascontiguousarray(t.astype(_BF16)).reshape(NFCH, 128, NFT * H)


def _core_blocks(c1: int, c2: int):
    # big blocks first in both cells: a full first block keeps the PE
    # streaming once its x lands (a small first block makes the ramp dribble
    # at DMA pace), and big-first in cell B leaves a small final block whose
    # stage B + write-out form the kernel tail.
    blocks = []
    off = 0
    for sz in _cell_blocks(c1, small_first=False):
        blocks.append((off, sz, 0))
        off += sz
    for sz in _cell_blocks(c2, small_first=False):
        blocks.append((off, sz, 1))
        off += sz
    return blocks


def kernel(x, router_w, w1, b1, w2, b2, w3, b3):
    from concourse.bass_utils import run_bass_kernel_spmd

    B, S, _ = x.shape
    T = B * S
    x2d = np.ascontiguousarray(x, dtype=np.float32).reshape(T, H)

    idx_e, cw_e = _route(x2d, np.asarray(router_w, dtype=np.float32))
    loads = [len(i) for i in idx_e]
    c1, c2, cells1, cells2 = _plan_cells(loads)
    cap = c1 + c2

    # token ranges per cell: experts consume their index lists in cell order
    # (cells1 scan order, then cells2) — must match _plan_cells's fill order.
    eoff = [0] * E
    core_cells = [[None, None] for _ in range(E)]
    for g, cells, ccap in ((0, cells1, c1), (1, cells2, c2)):
        for core, (e, n) in enumerate(cells):
            core_cells[core][g] = (e, eoff[e], n)
            eoff[e] += n
    for e in range(E):
        assert eoff[e] == loads[e], (e, eoff[e], loads[e])

    use_b2 = bool(np.any(b2))
    key = (c1, c2, use_b2)
    nc = _kernel_cache.get(key)
    if nc is None:
        nc = _build(c1, c2, use_b2)
        _kernel_cache[key] = nc

    # pack weights once per expert (in_maps share references)
    pw1 = [_pack_w12(w1[e]) for e in range(E)]
    pw2 = [_pack_w12(w2[e]) for e in range(E)]
    pw3 = [_pack_w3(w3[e]) for e in range(E)]
    pb1 = [
        np.ascontiguousarray(
            np.asarray(b1[e], dtype=np.float32).reshape(F // 128, 128).T
        )
        for e in range(E)
    ]
    pb3 = [
        np.ascontiguousarray(
            np.asarray(b3[e], dtype=np.float32).reshape(NHT, 128).T
        )
        for e in range(E)
    ]
    if use_b2:
        pb2 = [
            np.ascontiguousarray(
                np.asarray(b2[e], dtype=np.float32).reshape(F // 128, 128).T
            )
            for e in range(E)
        ]

    blocks = _core_blocks(c1, c2)
    cell_off = (0, c1)

    in_maps = []
    for core in range(E):
        # gather this core's tokens: cell A rows [0, c1), cell B rows [c1, cap)
        xg = np.zeros((cap, H), dtype=np.float32)
        for g in range(2):
            e, st, n = core_cells[core][g]
            if n:
                xg[cell_off[g]:cell_off[g] + n] = x2d[idx_e[e][st:st + n]]
        xb = xg.astype(_BF16)
        xTe = np.concatenate(
            [
                xb[off:off + sz].reshape(sz, NHT, 128)
                .transpose(2, 1, 0).reshape(128, NHT * sz)
                for off, sz, _ in blocks
            ],
            axis=1,
        )
        m = {"xT": np.ascontiguousarray(xTe)}
        for g in range(2):
            e = core_cells[core][g][0]
            m[f"w1{g}"] = pw1[e]
            m[f"w2{g}"] = pw2[e]
            m[f"w3{g}"] = pw3[e]
            m[f"b1{g}"] = pb1[e]
            m[f"b3{g}"] = pb3[e]
            if use_b2:
                m[f"b2{g}"] = pb2[e]
        in_maps.append(m)

    global _last_in_maps
    _last_in_maps = in_maps
    res = run_bass_kernel_spmd(nc, in_maps, core_ids=list(range(E)))

    out = np.zeros((T, H), dtype=np.float32)
    for core in range(E):
        yTe = np.asarray(res.results[core]["yT"], dtype=np.float32)
        for g in range(2):
            e, st, n = core_cells[core][g]
            if not n:
                continue
            co = cell_off[g]
            # per-block unpack: cols NHT*off + i*sz + t
            ye = np.empty((core_cells[core][g][2], H), dtype=np.float32)
            for off, sz, bg in blocks:
                if bg != g:
                    continue
                rel = off - co   # row range of this block within the cell
                if rel >= n:
                    continue
                take = min(sz, n - rel)
                blk = yTe[:, NHT * off:NHT * (off + sz)].reshape(128, NHT, sz)
                ye[rel:rel + take] = (
                    blk[:, :, :take].transpose(2, 1, 0).reshape(take, H)
                )
            idx = idx_e[e][st:st + n]
            out[idx] += ye * cw_e[e][st:st + n][:, None]
    return out.reshape(B, S, H)


# revision 18
# speedup vs baseline: 1.0150x; 1.0010x over previous
"""MoE FFN (top-2 of 8 experts, SwiGLU) for 8 Trainium2 NeuronCores.

Strategy: load-balanced expert parallelism. The router (tiny [T,H]@[H,E]
matmul + softmax + top-2) runs on host as part of sharding; the 16384
(token, expert) pairs are packed into 8 cores x 2 expert-cells of uniform
capacities (c1, c2) found by a small feasibility search, so every core gets
~2048 pairs instead of the max expert load (~2180). Each cell is bound to
one expert; the host supplies that expert's packed weights as the cell's
weight parameters (shared references, no extra packing). Each core runs a
dense SwiGLU FFN over its cells' tokens in bf16 (fp32 PSUM accumulation),
feature-on-partition / token-on-free-dim, weights streamed chunk-by-chunk
(chunk-major over both cells) so SBUF holds one f-chunk per cell turn.

Per-core device program per (f-chunk fc, cell g), blocks of <=512 tokens:
  g_T[f, t] = sum_i w1[h_i, f]^T @ x_T[h_i, t]        (PSUM accum over h-tiles)
  u_T[f, t] likewise with w2
  h_T[f, t] = silu(g_T + b1) * (u_T + b2)             (ACT + DVE, -> bf16)
  y_T[h, t] = sum_f w3[f, h]^T @ h_T[f, t] + b3       (PSUM accum per f-chunk,
                                                       accumulated in SBUF f32)
At the last chunk the accumulated y is emitted as bf16 and written back with
one fused DMA per block on the (otherwise idle) gpsimd SWDGE queue, so the
write-outs never block the weight-streaming queues. A short burst of warm-up
matmuls on a memset tile flips the PE HAM clock-gate to 8/8 before the first
real data lands, and the prologue DMAs are ordered so the first token block
and first w1/w2 pieces arrive as early as possible.
"""

import numpy as np
import ml_dtypes

E = 8       # experts
K = 2       # top-k
H = 1024    # hidden
F = 4096    # ffn dim
BLK = 512   # max tokens per block (moving free dim of every matmul)
FCH = 512   # f-chunk size (weight streaming granularity); FCH % 128 == 0

NHT = H // 128    # h-tiles
NFCH = F // FCH   # f-chunks
NFT = FCH // 128  # f-tiles per chunk

_BF16 = ml_dtypes.bfloat16

_kernel_cache: dict[object, object] = {}
_last_in_maps = None


def _cell_blocks(c: int, small_first: bool):
    """Decompose a cell capacity into token blocks of <=512."""
    r = c % BLK
    blocks = [BLK] * (c // BLK)
    if r:
        blocks = ([r] + blocks) if small_first else (blocks + [r])
    return blocks


def _plan_cells(loads: list[int]):
    """Find uniform cell capacities (c1 >= c2) and an assignment of experts
    to the 8 c1-cells + 8 c2-cells minimizing cap = c1 + c2.

    Returns (c1, c2, cells1, cells2) where cells1/cells2 are length-8 lists
    of (expert, n_tokens) per core (n_tokens may be 0 for unused cells)."""
    order = sorted(range(E), key=lambda e: -loads[e])

    def try_fit(c1, c2):
        # DFS over experts (desc load): pick (a, b) cells with
        # a*c1 + b*c2 >= load, total a <= 8, b <= 8.
        picks = {}

        def dfs(i, a_left, b_left):
            if i == len(order):
                return True
            L = loads[order[i]]
            cands = []
            for a in range(0, a_left + 1):
                rem = L - a * c1
                b = 0 if rem <= 0 else -(-rem // c2)
                if b <= b_left:
                    cands.append((a + b, a, b))
            cands.sort()
            for _, a, b in cands:
                picks[order[i]] = (a, b)
                if dfs(i + 1, a_left - a, b_left - b):
                    return True
            picks.pop(order[i], None)
            return False

        return picks if dfs(0, E, E) else None

    for cap in range(2048, 2048 + 1024, 16):
        lo = (cap + 1) // 2
        lo = -(-lo // 16) * 16
        for c1 in range(lo, cap - 255, 16):
            c2 = cap - c1
            if c2 < 256 or c2 > c1:
                continue
            picks = try_fit(c1, c2)
            if picks is not None:
                # materialize cells: assign expert cells to cores in order
                cells1, cells2 = [], []
                for e in order:
                    a, b = picks[e]
                    rem = loads[e]
                    for _ in range(a):
                        n = min(rem, c1)
                        cells1.append((e, n))
                        rem -= n
                    for _ in range(b):
                        n = min(rem, c2)
                        cells2.append((e, n))
                        rem -= n
                while len(cells1) < E:
                    cells1.append((0, 0))
                while len(cells2) < E:
                    cells2.append((0, 0))
                return c1, c2, cells1, cells2
    raise RuntimeError("no feasible cell plan found")


def _build(c1: int, c2: int, use_b2: bool):
    """Build the per-core Bass/Tile program for cell capacities (c1, c2)."""
    import concourse.bass as bass  # noqa: F401
    import concourse.tile as tile
    from concourse import bacc, mybir

    bf16 = mybir.dt.bfloat16
    f32 = mybir.dt.float32
    AF = mybir.ActivationFunctionType

    cap = c1 + c2
    blocks = _core_blocks(c1, c2)

    nc = bacc.Bacc("TRN2", target_bir_lowering=False, debug=False, num_devices=E)

    xT = nc.declare_dram_parameter("xT", [128, NHT * cap], bf16, isOutput=False)
    wps = []  # weight params per group: (w1, w2, w3)
    bps = []  # bias params per group: (b1, b3) or (b1, b2, b3)
    for g in range(2):
        w1 = nc.declare_dram_parameter(f"w1{g}", [NFCH, 128, NFT * H], bf16, isOutput=False)
        w2 = nc.declare_dram_parameter(f"w2{g}", [NFCH, 128, NFT * H], bf16, isOutput=False)
        w3 = nc.declare_dram_parameter(f"w3{g}", [NFCH, 128, NFT * H], bf16, isOutput=False)
        wps.append((w1, w2, w3))
        b1 = nc.declare_dram_parameter(f"b1{g}", [128, F // 128], f32, isOutput=False)
        b3 = nc.declare_dram_parameter(f"b3{g}", [128, NHT], f32, isOutput=False)
        if use_b2:
            b2 = nc.declare_dram_parameter(f"b2{g}", [128, F // 128], f32, isOutput=False)
            bps.append((b1, b2, b3))
        else:
            bps.append((b1, b3))
    yT = nc.declare_dram_parameter("yT", [128, NHT * cap], bf16, isOutput=True)

    with tile.TileContext(nc) as tc:
        with (
            tc.tile_pool(name="xp", bufs=1) as xp,
            tc.tile_pool(name="yp", bufs=1) as yp,
            tc.tile_pool(name="wp", bufs=2) as wp,
            tc.tile_pool(name="hp", bufs=2) as hp,
            tc.tile_pool(name="sp", bufs=3) as sp,
            tc.tile_pool(name="pg", bufs=2, space="PSUM") as pg,
            tc.tile_pool(name="pu", bufs=2, space="PSUM") as pu,
            tc.tile_pool(name="py", bufs=2, space="PSUM") as py,
        ):
            op = hp  # write-out tiles share the hp pool (fewer pools)
            # ---- HAM warm-up: keep the PE busy from ~4us so the clock-gate
            # is at 8/8 when the first real matmul issues. No data deps.
            warm = xp.tile([128, 512], bf16, name="warm")
            nc.vector.memset(warm[:], 0)
            psw = py.tile([128, 512], f32, tag="y", name="psw")
            NWARM = 12
            for k in range(NWARM):
                nc.tensor.matmul(
                    psw[:], warm[:, 0:128], warm[:],
                    start=(k == 0), stop=(k == NWARM - 1),
                )

            def pads(src, n):
                # HAM keep-alive matmuls tied to a just-landed prologue
                # piece: they keep PE duty high through the DMA-paced ramp
                # so the clock-gate stays at 8/8 (results unused).
                for k in range(n):
                    nc.tensor.matmul(psw[:], warm[:, 0:128], src,
                                     start=True, stop=True)

            # ---- resident tiles
            # Tokens (bf16): block-major columns — block at global offset
            # `off` spans cols [NHT*off, NHT*(off+sz)), h-tile i contiguous
            # inside it (col = NHT*off + i*sz + t). Host supplies identical
            # layout: each block is ONE contiguous 2D transfer.
            xall = xp.tile([128, NHT * cap], bf16, name="xall")

            def xsl(i, off, sz):
                base = NHT * off + i * sz
                return xall[:, base:base + sz]

            # f32 accumulator for chunks 0..NFCH-2, h-tile-major columns.
            yall = yp.tile([128, NHT * cap], f32, name="yall")

            def ysl(i, off, sz):
                return yall[:, i * cap + off:i * cap + off + sz]

            # ---- prologue DMAs. The first matmul group needs x of block 0
            # AND w1's first piece; both queues pull concurrently from a
            # shared per-core DMA bandwidth pool, so split x block 0 across
            # both queues and interleave the w1/w2 chunk-0 quarter pieces so
            # the PE can start ~2us after the critical 1.5MB lands.
            o0, s0, _ = blocks[0]
            mid = NHT * o0 + (NHT // 2) * s0
            end0 = NHT * (o0 + s0)
            # scalar q: x block-0 low half, then the remaining blocks
            nc.scalar.dma_start(xall[:, NHT * o0:mid], xT[:, NHT * o0:mid])
            pads(xall[:, NHT * o0:NHT * o0 + 512], 4)
            for o, s, g in blocks[1:]:
                lo, hi = NHT * o, NHT * (o + s)
                nc.scalar.dma_start(xall[:, lo:hi], xT[:, lo:hi])

            # sync q: w1 first quarter, x block-0 high half, w2 first
            # quarter, biases, remaining quarters, w3.
            bts = [None, None]
            w1cA = wp.tile([128, NFT * H], bf16, tag="w1", name="w1c")
            w2cA = wp.tile([128, NFT * H], bf16, tag="w2", name="w2c")
            nc.sync.dma_start(w1cA[:, 0:H], wps[0][0][0][:, 0:H])
            pads(w1cA[:, 0:512], 2)
            nc.sync.dma_start(xall[:, mid:end0], xT[:, mid:end0])
            pads(xall[:, mid:mid + 512], 4)
            nc.sync.dma_start(w2cA[:, 0:H], wps[0][1][0][:, 0:H])
            pads(w2cA[:, 0:512], 2)
            b1t = xp.tile([128, F // 128], f32, name="b1t0")
            nc.sync.dma_start(b1t[:], bps[0][0][:])
            b3t = xp.tile([128, NHT], f32, name="b3t0")
            nc.sync.dma_start(b3t[:], bps[0][-1][:])
            if use_b2:
                b2t = xp.tile([128, F // 128], f32, name="b2t0")
                nc.sync.dma_start(b2t[:], bps[0][1][:])
                bts[0] = (b1t, b2t, b3t)
            else:
                bts[0] = (b1t, b3t)
            for j in range(1, NFT):
                jsl = slice(j * H, (j + 1) * H)
                nc.sync.dma_start(w1cA[:, jsl], wps[0][0][0][:, jsl])
                nc.sync.dma_start(w2cA[:, jsl], wps[0][1][0][:, jsl])
            w3cA = wp.tile([128, NFT * H], bf16, tag="w3", name="w3c")
            nc.sync.dma_start(w3cA[:], wps[0][2][0])
            # cell-B biases
            b1t = xp.tile([128, F // 128], f32, name="b1t1")
            nc.sync.dma_start(b1t[:], bps[1][0][:])
            b3t = xp.tile([128, NHT], f32, name="b3t1")
            nc.sync.dma_start(b3t[:], bps[1][-1][:])
            if use_b2:
                b2t = xp.tile([128, F // 128], f32, name="b2t1")
                nc.sync.dma_start(b2t[:], bps[1][1][:])
                bts[1] = (b1t, b2t, b3t)
            else:
                bts[1] = (b1t, b3t)

            def stage_b(fc, grp, off, sz, ht_tiles, w3t):
                b3t = bts[grp][-1]
                yo = None
                if fc == NFCH - 1:
                    yo = op.tile([128, NHT * sz], bf16, tag="yo", name="yo")
                for i in range(NHT):
                    psy = py.tile([128, sz], f32, tag="y", name="psy")
                    for j in range(NFT):
                        nc.tensor.matmul(
                            psy[:],
                            w3t[:, j * H + i * 128:j * H + (i + 1) * 128],
                            ht_tiles[j][:],
                            start=(j == 0), stop=(j == NFT - 1),
                        )
                    if fc == 0:
                        nc.scalar.activation(
                            ysl(i, off, sz), psy[:], AF.Identity,
                            bias=b3t[:, i:i + 1],
                        )
                    elif fc < NFCH - 1:
                        nc.vector.tensor_add(
                            ysl(i, off, sz), ysl(i, off, sz), psy[:]
                        )
                    else:
                        nc.vector.tensor_add(
                            yo[:, i * sz:(i + 1) * sz], ysl(i, off, sz), psy[:]
                        )
                        if i % 2 == 1:
                            # write out per pair of h-tiles as soon as the
                            # data is ready; sync queue is safe here because
                            # both groups' weight loads for this chunk were
                            # issued before any write-out (no cycle).
                            l, r = (i - 1) * sz, (i + 1) * sz
                            nc.sync.dma_start(
                                yT[:, NHT * off + l:NHT * off + r],
                                yo[:, l:r],
                            )

            pending = None
            for fc in range(NFCH):
                # load this chunk's weights for BOTH cells up front (tag
                # alternation keeps the bufs=2 double-buffering intact)
                wt = [None, None]
                wt[0] = (w1cA, w2cA, w3cA) if fc == 0 else None
                if wt[0] is None:
                    w1c = wp.tile([128, NFT * H], bf16, tag="w1", name="w1c")
                    nc.sync.dma_start(w1c[:], wps[0][0][fc])
                    w2c = wp.tile([128, NFT * H], bf16, tag="w2", name="w2c")
                    nc.sync.dma_start(w2c[:], wps[0][1][fc])
                    w3c = wp.tile([128, NFT * H], bf16, tag="w3", name="w3c")
                    nc.sync.dma_start(w3c[:], wps[0][2][fc])
                    wt[0] = (w1c, w2c, w3c)
                w1c = wp.tile([128, NFT * H], bf16, tag="w1", name="w1c")
                nc.sync.dma_start(w1c[:], wps[1][0][fc])
                w2c = wp.tile([128, NFT * H], bf16, tag="w2", name="w2c")
                nc.sync.dma_start(w2c[:], wps[1][1][fc])
                w3c = wp.tile([128, NFT * H], bf16, tag="w3", name="w3c")
                nc.sync.dma_start(w3c[:], wps[1][2][fc])
                wt[1] = (w1c, w2c, w3c)

                for grp in range(2):
                    w1c, w2c, w3c = wt[grp]
                    b1t = bts[grp][0]
                    for off, sz, g in blocks:
                        if g != grp:
                            continue
                        # Stage A: h_T[f, tok] = silu(g_T + b1) * (u_T + b2)
                        ht_tiles = []
                        for j in range(NFT):
                            fg = fc * NFT + j
                            psg = pg.tile([128, sz], f32, tag="g", name="psg")
                            for i in range(NHT):
                                base = (j * NHT + i) * 128
                                nc.tensor.matmul(
                                    psg[:], w1c[:, base:base + 128],
                                    xsl(i, off, sz),
                                    start=(i == 0), stop=(i == NHT - 1),
                                )
                            s = sp.tile([128, sz], f32, tag="s", name="stile")
                            nc.scalar.activation(
                                s[:], psg[:], AF.Silu, bias=b1t[:, fg:fg + 1]
                            )
                            psu = pu.tile([128, sz], f32, tag="u", name="psu")
                            for i in range(NHT):
                                base = (j * NHT + i) * 128
                                nc.tensor.matmul(
                                    psu[:], w2c[:, base:base + 128],
                                    xsl(i, off, sz),
                                    start=(i == 0), stop=(i == NHT - 1),
                                )
                            h = hp.tile([128, sz], bf16, tag=f"h{j}", name=f"htile{j}")
                            if use_b2:
                                b2t = bts[grp][1]
                                u2 = sp.tile([128, sz], f32, tag="u2", name="u2tile")
                                nc.scalar.activation(
                                    u2[:], psu[:], AF.Identity,
                                    bias=b2t[:, fg:fg + 1]
                                )
                                nc.vector.tensor_mul(h[:], s[:], u2[:])
                            else:
                                nc.vector.tensor_mul(h[:], s[:], psu[:])
                            ht_tiles.append(h)

                        if pending is not None:
                            stage_b(*pending)
                        pending = (fc, grp, off, sz, ht_tiles, w3c)
            stage_b(*pending)

    nc.finalize()
    return nc


def _route(x2d: np.ndarray, router_w: np.ndarray):
    """Host router: softmax over experts, top-2. Returns per-expert token
    index lists and combine weights."""
    logits = x2d @ router_w                       # [T, E]
    logits -= logits.max(axis=-1, keepdims=True)
    p = np.exp(logits, dtype=np.float32)
    p /= p.sum(axis=-1, keepdims=True)
    order = np.argsort(-p, axis=-1, kind="stable")[:, :K]   # [T, K]
    idx_e, cw_e = [], []
    for e in range(E):
        sel = np.nonzero((order == e).any(axis=1))[0]
        idx_e.append(sel)
        cw_e.append(p[sel, e])
    return idx_e, cw_e


def _pack_w12(w: np.ndarray) -> np.ndarray:
    """[H, F] f32 -> [NFCH, 128, NFT*NHT*128] bf16 with column order (j, i, q):
    chunk c, partition p, f-tile j, h-tile i, col q = w[i*128+p, c*FCH+j*128+q].
    """
    t = np.asarray(w, dtype=np.float32).reshape(NHT, 128, NFCH, NFT, 128)
    t = t.transpose(2, 1, 3, 0, 4)  # [c, p, j, i, q]
    return np.ascontiguousarray(t.astype(_BF16)).reshape(NFCH, 128, NFT * H)


def _pack_w3(w: np.ndarray) -> np.ndarray:
    """[F, H] f32 -> [NFCH, 128, NFT*H] bf16 with column order (j, h):
    chunk c, partition p (= f within f-tile j) -> w[c*FCH+j*128+p, h]."""
    t = np.asarray(w, dtype=np.float32).reshape(NFCH, NFT, 128, H)
    t = t.transpose(0, 2, 1, 3)  # [c, p, j, h]
    return np.ascontiguousarray(t.astype(_BF16)).reshape(NFCH, 128, NFT * H)


def _core_blocks(c1: int, c2: int):
    # big blocks first in both cells: a full first block keeps the PE
    # streaming once its x lands (a small first block makes the ramp dribble
    # at DMA pace), and big-first in cell B leaves a small final block whose
    # stage B + write-out form the kernel tail.
    blocks = []
    off = 0
    for sz in _cell_blocks(c1, small_first=False):
        blocks.append((off, sz, 0))
        off += sz
    for sz in _cell_blocks(c2, small_first=False):
        blocks.append((off, sz, 1))
        off += sz
    return blocks


def kernel(x, router_w, w1, b1, w2, b2, w3, b3):
    from concourse.bass_utils import run_bass_kernel_spmd

    B, S, _ = x.shape
    T = B * S
    x2d = np.ascontiguousarray(x, dtype=np.float32).reshape(T, H)

    idx_e, cw_e = _route(x2d, np.asarray(router_w, dtype=np.float32))
    loads = [len(i) for i in idx_e]
    c1, c2, cells1, cells2 = _plan_cells(loads)
    cap = c1 + c2

    # token ranges per cell: experts consume their index lists in cell order
    # (cells1 scan order, then cells2) — must match _plan_cells's fill order.
    eoff = [0] * E
    core_cells = [[None, None] for _ in range(E)]
    for g, cells, ccap in ((0, cells1, c1), (1, cells2, c2)):
        for core, (e, n) in enumerate(cells):
            core_cells[core][g] = (e, eoff[e], n)
            eoff[e] += n
    for e in range(E):
        assert eoff[e] == loads[e], (e, eoff[e], loads[e])

    use_b2 = bool(np.any(b2))
    key = (c1, c2, use_b2)
    nc = _kernel_cache.get(key)
    if nc is None:
        nc = _build(c1, c2, use_b2)
        _kernel_cache[key] = nc

    # pack weights once per expert (in_maps share references)
    pw1 = [_pack_w12(w1[e]) for e in range(E)]
    pw2 = [_pack_w12(w2[e]) for e in range(E)]
    pw3 = [_pack_w3(w3[e]) for e in range(E)]
    pb1 = [
        np.ascontiguousarray(
            np.asarray(b1[e], dtype=np.float32).reshape(F // 128, 128).T
        )
        for e in range(E)
    ]
    pb3 = [
        np.ascontiguousarray(
            np.asarray(b3[e], dtype=np.float32).reshape(NHT, 128).T
        )
        for e in range(E)
    ]
    if use_b2:
        pb2 = [
            np.ascontiguousarray(
                np.asarray(b2[e], dtype=np.float32).reshape(F // 128, 128).T
            )
            for e in range(E)
        ]

    blocks = _core_blocks(c1, c2)
    cell_off = (0, c1)

    in_maps = []
    for core in range(E):
        # gather this core's tokens: cell A rows [0, c1), cell B rows [c1, cap)
        xg = np.zeros((cap, H), dtype=np.float32)
        for g in range(2):
            e, st, n = core_cells[core][g]
            if n:
                xg[cell_off[g]:cell_off[g] + n] = x2d[idx_e[e][st:st + n]]
        xb = xg.astype(_BF16)
        xTe = np.concatenate(
            [
                xb[off:off + sz].reshape(sz, NHT, 128)
                .transpose(2, 1, 0).reshape(128, NHT * sz)
                for off, sz, _ in blocks
            ],
            axis=1,
        )
        m = {"xT": np.ascontiguousarray(xTe)}
        for g in range(2):
            e = core_cells[core][g][0]
            m[f"w1{g}"] = pw1[e]
            m[f"w2{g}"] = pw2[e]
            m[f"w3{g}"] = pw3[e]
            m[f"b1{g}"] = pb1[e]
            m[f"b3{g}"] = pb3[e]
            if use_b2:
                m[f"b2{g}"] = pb2[e]
        in_maps.append(m)

    global _last_in_maps
    _last_in_maps = in_maps
    res = run_bass_kernel_spmd(nc, in_maps, core_ids=list(range(E)))

    out = np.zeros((T, H), dtype=np.float32)
    for core in range(E):
        yTe = np.asarray(res.results[core]["yT"], dtype=np.float32)
        for g in range(2):
            e, st, n = core_cells[core][g]
            if not n:
                continue
            co = cell_off[g]
            # per-block unpack: cols NHT*off + i*sz + t
            ye = np.empty((core_cells[core][g][2], H), dtype=np.float32)
            for off, sz, bg in blocks:
                if bg != g:
                    continue
                rel = off - co   # row range of this block within the cell
                if rel >= n:
                    continue
                take = min(sz, n - rel)
                blk = yTe[:, NHT * off:NHT * (off + sz)].reshape(128, NHT, sz)
                ye[rel:rel + take] = (
                    blk[:, :, :take].transpose(2, 1, 0).reshape(take, H)
                )
            idx = idx_e[e][st:st + n]
            out[idx] += ye * cw_e[e][st:st + n][:, None]
    return out.reshape(B, S, H)
